# revision 7
# baseline (speedup 1.0000x reference)
"""Trainium2 Bass kernel for DGATNet DiffusioUnpool (gnn_message_passing).

Math (reference):
    z = scatter(fea|win_nac by perm into N rows)      # N=12288, 33 cols
    A = scatter_add(edges) + I                        # dense adjacency
    d = 1/sqrt(rowsum(A))
    out = d * (A @ (d * z))                           # x: cols 0:32, atte: col 32

Device strategy (8 NeuronCores, row-sharded):
    - Edges bucketed by destination row (1536 rows/core), duplicate edges
      pre-summed, rows degree-sorted, packed into per-128-row-block ELL slot
      grids (idx = source node, w = edge weight; self-loop = extra slot, w=1).
    - Full-node degree computed on every core from a replicated natural-order
      weight-slot grid (one tensor_reduce) -> d = 1/sqrt(deg); no collective.
    - u = d * z built on-device: dma_gather rows of [fea|win_nac] by inverse
      perm, multiplied by d, stored to an HBM table u_pad [N, 64] (256B rows).
    - Main pass per block: one dma_gather of u_pad rows by edge source index
      (one 256B row per slot), then per-slot fused multiply-add on VectorE:
      y[p,:] += w[p,k] * G[p,k,:33]; final scale by own-row d; DMA out.
    - Host only buckets/sorts/pads indices (sharding prep) and re-assembles
      the row-sharded, row-permuted outputs.
"""

import numpy as np

N = 12288
NC = 8
RPC = N // NC          # rows per core = 1536
P = 128
NB = RPC // P          # 12 blocks per core
NJ = N // P            # 96 natural-layout columns
FC = 33                # feature cols: 32 fea + 1 atte
ELEM = 64              # u_pad row: 64 f32 = 256 B
NPERM = 6144
FEXT_ROWS = 6272       # fea_ext rows (6144 + zero row + pad)

_cache = {}


def _build_program(K_blocks, sk, kd, num_devices=NC, repeat=1, loop=1):
    import contextlib
    import concourse.bacc as bacc
    import concourse.mybir as mybir
    import concourse.tile as tile

    f32 = mybir.dt.float32
    i16 = mybir.dt.int16
    woff = np.concatenate([[0], np.cumsum(K_blocks)]).astype(int)

    nc = bacc.Bacc("TRN2", target_bir_lowering=False, debug=False,
                   num_devices=num_devices)
    t_fext = nc.dram_tensor("fext", [FEXT_ROWS, ELEM], f32, kind="ExternalInput").ap()
    t_fidx = nc.dram_tensor("fidx", [P, N // 16], i16, kind="ExternalInput").ap()
    t_idx = nc.dram_tensor("idx", [P, 8 * sk], i16, kind="ExternalInput").ap()
    t_w = nc.dram_tensor("w", [P, sk], f32, kind="ExternalInput").ap()
    t_wdeg = nc.dram_tensor("wdeg", [P, NJ * kd], f32, kind="ExternalInput").ap()
    t_x = nc.dram_tensor("out_x", [RPC, 32], f32, kind="ExternalOutput").ap()
    t_a = nc.dram_tensor("out_atte", [RPC, 1], f32, kind="ExternalOutput").ap()

    with tile.TileContext(nc) as tc:
        with (
            tc.tile_pool(name="persist", bufs=1) as pp,
            tc.tile_pool(name="gbuf", bufs=3) as gp,
            tc.tile_pool(name="ybuf", bufs=2) as yp,
            tc.tile_pool(name="dram", bufs=1, space="DRAM") as dp,
        ):
            loop_cm = tc.For_i(0, loop, 1) if loop > 1 else contextlib.nullcontext()
            with loop_cm:
              for rep in range(repeat):
                s = f"_{rep}" if repeat > 1 else ""
                # ---- load slot data / indices ----
                fidx_sb = pp.tile([P, N // 16], i16, name=f"fidx_sb{s}", tag="fidx_sb")
                nc.sync.dma_start(fidx_sb[:], t_fidx[:])
                idx_sb = pp.tile([P, 8 * sk], i16, name=f"idx_sb{s}", tag="idx_sb")
                nc.sync.dma_start(idx_sb[:], t_idx[:])
                w_sb = pp.tile([P, sk], f32, name=f"w_sb{s}", tag="w_sb")
                nc.sync.dma_start(w_sb[:], t_w[:])
                wdeg_sb = pp.tile([P, NJ * kd], f32, name=f"wdeg_sb{s}", tag="wdeg_sb")
                nc.sync.dma_start(wdeg_sb[:], t_wdeg[:])

                # ---- z gather: z[p, j, :] = fea_ext[inv_perm[node(p,j)]] ----
                zg = pp.tile([P, NJ, ELEM], f32, name=f"zg{s}", tag="zg")
                nc.gpsimd.dma_gather(
                    zg[:], t_fext[:], fidx_sb[:], N, N, ELEM, single_packet=False
                )

                # ---- full deg -> d_sb [128, 96] (natural node = p*96 + j) ----
                degn = pp.tile([P, NJ], f32, name=f"degn{s}", tag="degn")
                nc.vector.reduce_sum(
                    degn[:],
                    wdeg_sb.rearrange("p (j k) -> p j k", k=kd),
                    axis=mybir.AxisListType.X,
                )
                sqn = pp.tile([P, NJ], f32, name=f"sqn{s}", tag="sqn")
                nc.scalar.sqrt(sqn[:], degn[:])
                d_sb = pp.tile([P, NJ], f32, name=f"d_sb{s}", tag="d_sb")
                nc.vector.reciprocal(d_sb[:], sqn[:])

                # ---- own-row d (sorted order) for the final scale ----
                deg = pp.tile([P, NB], f32, name=f"deg{s}", tag="deg")
                for b in range(NB):
                    nc.vector.reduce_sum(
                        deg[:, b : b + 1],
                        w_sb[:, woff[b] : woff[b + 1]],
                        axis=mybir.AxisListType.X,
                    )
                sq = pp.tile([P, NB], f32, name=f"sq{s}", tag="sq")
                nc.scalar.sqrt(sq[:], deg[:])
                dpi = pp.tile([P, NB], f32, name=f"dpi{s}", tag="dpi")
                nc.vector.reciprocal(dpi[:], sq[:])

                # ---- u = d * z ; u_pad [N, 64] (node = p*96 + j) ----
                nc.vector.tensor_tensor(
                    out=zg[:, :, : FC + 1],
                    in0=zg[:, :, : FC + 1],
                    in1=d_sb[:, :, None].to_broadcast([P, NJ, FC + 1]),
                    op=mybir.AluOpType.mult,
                )
                u_pad = dp.tile([N, ELEM], f32, name=f"u_pad{s}", tag="u_pad")
                nc.sync.dma_start(u_pad.rearrange("(p j) e -> p j e", p=P), zg[:])

                # ---- main: per block gather + FMA + scale + out ----
                for b in range(NB):
                    kb = int(K_blocks[b])
                    g = gp.tile([P, kb, ELEM], f32, tag="g", name=f"g{b}{s}")
                    nc.gpsimd.dma_gather(
                        g[:], u_pad[:],
                        idx_sb[:, 8 * woff[b] : 8 * woff[b + 1]],
                        P * kb, P * kb, ELEM, single_packet=False,
                    )
                    nc.vector.tensor_tensor(
                        out=g[:, :, :FC],
                        in0=g[:, :, :FC],
                        in1=w_sb[:, woff[b] : woff[b + 1], None].to_broadcast(
                            [P, kb, FC]
                        ),
                        op=mybir.AluOpType.mult,
                    )
                    y = yp.tile([P, FC], f32, tag="y", name=f"y{b}{s}")
                    nc.vector.reduce_sum(
                        y[:], g[:, :, :FC].transpose((0, 2, 1)),
                        axis=mybir.AxisListType.X,
                    )
                    o = yp.tile([P, FC], f32, tag="o", name=f"o{b}{s}")
                    nc.vector.tensor_scalar_mul(o[:], y[:], dpi[:, b : b + 1])
                    nc.sync.dma_start(t_x[b * P : (b + 1) * P, :], o[:, :32])
                    nc.sync.dma_start(t_a[b * P : (b + 1) * P, :], o[:, 32:33])

    nc.compile()
    return nc


def _wrap16(lst):
    """Gather index list -> [16, n/16] wrapped layout (pos i at [i%16, i//16])."""
    return np.ascontiguousarray(lst.reshape(-1, 16).T)


def _prep(fea, perm, ei, ew, coffe):
    r = ei[0].astype(np.int64)
    c = ei[1].astype(np.int64)
    w = ew.astype(np.float64)
    # self loops (A + I), then combine duplicate (r, c) pairs (scatter-add)
    r = np.concatenate([r, np.arange(N, dtype=np.int64)])
    c = np.concatenate([c, np.arange(N, dtype=np.int64)])
    w = np.concatenate([w, np.ones(N, np.float64)])
    uniq, inv_u = np.unique(r * N + c, return_inverse=True)
    w = np.bincount(inv_u, weights=w).astype(np.float32)
    r = uniq // N
    c = uniq % N

    counts = np.bincount(r, minlength=N)
    rank = np.empty(N, np.int64)
    pi_all = np.empty((NC, RPC), np.int64)
    for m in range(NC):
        pi = np.argsort(-counts[m * RPC : (m + 1) * RPC], kind="stable")
        pi_all[m] = pi
        rank[m * RPC + pi] = np.arange(RPC)

    Kb_mc = np.empty((NC, NB), np.int64)
    for m in range(NC):
        cs_m = counts[m * RPC + pi_all[m]]
        Kb_mc[m] = cs_m.reshape(NB, P).max(axis=1)
    K_blocks = Kb_mc.max(axis=0)
    woff = np.concatenate([[0], np.cumsum(K_blocks)]).astype(int)
    sk = int(woff[-1])
    kd = int(counts.max())

    # per-edge coordinates (rows already sorted in `r` after unique)
    row_start = np.concatenate([[0], np.cumsum(counts)])[:-1]
    slot = np.arange(len(r)) - row_start[r]
    m_e = r // RPC
    srt = rank[r]
    col_e = woff[srt // P] + slot
    p_e = srt % P

    idx_grids = np.zeros((NC, P, sk), np.int16)
    w_grids = np.zeros((NC, P, sk), np.float32)
    idx_grids[m_e, p_e, col_e] = c.astype(np.int16)
    w_grids[m_e, p_e, col_e] = w

    idx_all = np.empty((NC, 16, 8 * sk), np.int16)
    for m in range(NC):
        parts = [
            _wrap16(idx_grids[m][:, woff[b] : woff[b + 1]].T.ravel())
            for b in range(NB)
        ]
        idx_all[m] = np.concatenate(parts, axis=1)
    idx_all = np.tile(idx_all, (1, 8, 1))  # [NC, 128, 8*sk]

    # replicated natural-order weight slots for the full-deg reduce
    wdeg = np.zeros((P, NJ, kd), np.float32)
    wdeg[r // NJ, r % NJ, slot] = w

    # fea_ext + inverse perm gather indices (node layout p*96 + j)
    invp = np.full(N, NPERM, np.int64)
    invp[perm.astype(np.int64)] = np.arange(NPERM)
    nac = np.broadcast_to(
        coffe.reshape(8, 1, 192).astype(np.float32), (8, 4, 192)
    ).reshape(-1)
    fea_ext = np.zeros((FEXT_ROWS, ELEM), np.float32)
    fea_ext[:NPERM, :32] = fea
    fea_ext[:NPERM, 32] = nac
    i = np.arange(N)
    node = (i % P) * NJ + i // P
    fidx = np.tile(_wrap16(invp[node].astype(np.int16)), (8, 1))  # [128, 768]

    return K_blocks, sk, kd, fea_ext, fidx, idx_all, w_grids, wdeg, pi_all


def kernel(fea, perm, encoder_edge_index, encoder_edge_attr, node_atte_coffe,
           all_node_num=None, batch_size=None, **_unused):
    from concourse.bass_utils import run_bass_kernel_spmd

    fea = np.asarray(fea, np.float32)
    perm = np.asarray(perm)
    ei = np.asarray(encoder_edge_index)
    ew = np.asarray(encoder_edge_attr, np.float32)
    coffe = np.asarray(node_atte_coffe, np.float32)

    (K_blocks, sk, kd, fea_ext, fidx, idx_all, w_grids, wdeg, pi_all) = _prep(
        fea, perm, ei, ew, coffe
    )

    key = (tuple(int(k) for k in K_blocks), kd)
    if key not in _cache:
        _cache[key] = _build_program(K_blocks, sk, kd)
    nc = _cache[key]

    wdeg_flat = wdeg.reshape(P, NJ * kd)
    in_maps = [
        {
            "fext": fea_ext,
            "fidx": fidx,
            "idx": idx_all[m],
            "w": w_grids[m],
            "wdeg": wdeg_flat,
        }
        for m in range(NC)
    ]
    res = run_bass_kernel_spmd(nc, in_maps, core_ids=list(range(NC)))

    x = np.empty((N, 32), np.float32)
    atte = np.empty(N, np.float32)
    for m in range(NC):
        x[m * RPC + pi_all[m]] = res.results[m]["out_x"]
        atte[m * RPC + pi_all[m]] = res.results[m]["out_atte"][:, 0]
    return x, atte
